# revision 36
# baseline (speedup 1.0000x reference)
"""Trainium2 Bass kernel for LSPM (nn_LSPM_41455024341635).

Math refactor (same identities as the validated baseline):
  - scores = xf^T xf and softmax(scores) are identical for all 4 LSPM scales
    -> computed once per sample.
  - softmax shift uses diag[n] = ||x_n||^2 (shift-invariant, cancels exactly
    after row normalization).
  - row normalization folds into the mm2 lhs: xcs[n,s] = xcT[n,s]/rowsum[n].
  - all 1x1 convs + residuals fold into the output head:
      out = Wsum @ xf + h_all @ mm2,  Wsum = sum of w_final C-blocks,
      h_S = W_S @ relu(w_gap_S @ poolsum_S / win_S), h_all = concat_S h_S.
  - adaptive pools are raw window SUMS on device; the 1/win_S scale is folded
    into w_gap on the host. pool1/2/3 derive from pool6 block sums.

Sharding (collective-free): 8 cores = 4 samples x 2 attention ROW-halves.
Each core computes softmax rows n in its half (rowsum is local -> no
cross-core reduction) and produces a FULL-WIDTH partial output
  Y_h = [Wsum @ xf](own column half) + h_all @ mm2_h          [C, N] bf16
where mm2_h sums over the core's rows only. The host adds the two partials
(the head is linear in mm2). The row-half is presented to the one shared
program by pre-rolling x columns on the host (h=1 cores see x rolled by
N/2); pools read a separate canonical copy xp since a 24-row roll is not
pool3-window aligned. Host un-rolls the partial outputs before adding.

All matmuls run in bf16 (1 row/cycle on PE); exp on the Act engine per
512-col PSUM region; xcT is folded into the last scores region (free on PE).
"""

import os
import sys
import numpy as np

for _p in ("/opt/trn_rl_repo", "/root/.axon_site/_ro/trn_rl_repo"):
    if os.path.isdir(_p) and _p not in sys.path:
        sys.path.insert(0, _p)

import concourse.bass as bass
import concourse.bacc as bacc
import concourse.mybir as mybir
import concourse.tile as tile
from concourse import bass_utils

dt = mybir.dt
AF = mybir.ActivationFunctionType
AX = mybir.AxisListType

B, C, H, W = 4, 256, 48, 48
N = H * W            # 2304
HLOC = N // 2        # 1152 local attention rows per core
NBL = HLOC // 128    # 9 local row blocks
S2TOT = 50
SCALES = ((1, 1, 0), (2, 4, 1), (3, 9, 5), (6, 36, 14))  # (S, S2, col offset)
XWC = N + S2TOT      # 2354: x columns + folded wattnT columns
# scores regions per block: 4 x 512 + tail 306 (256 scores + 50 xcT)
REGS = ((0, 512, 512), (512, 512, 512), (1024, 512, 512), (1536, 512, 512),
        (2048, 306, 256))  # (col0, matmul width, exp width)
MMREGS = ((0, 512), (512, 512), (1024, 512), (1536, 512), (2048, 256))


def build_lspm(tc, outs, ins):
    nc = tc.nc
    xw_d, xp_d = ins["xw"], ins["xp"]
    wgapT_d, wT_d, wsumT_d = ins["wgapT"], ins["wT"], ins["wsumT"]
    out_d = outs["out"]
    bf = dt.bfloat16

    from contextlib import ExitStack
    with ExitStack() as ctx:
        pool = lambda name, bufs: ctx.enter_context(
            tc.tile_pool(name=name, bufs=bufs))
        sb_x = pool("x", 1)
        sb_e = pool("e", 1)
        sb_w = pool("w", 1)
        sb_s = pool("s", 1)
        sb_o = pool("o", 1)

        # ---- input DMAs ----
        # critical x loads trigger from the SP queue; the rest from the Act
        # queue (each trigger costs ~0.6-1.6us on its issuing sequencer, and
        # only SP/Act/Pool may trigger DMAs)
        xw_t = [sb_x.tile([128, XWC], bf, tag="xw", name="xw", bufs=2) for _ in range(2)]
        xp_t = [sb_x.tile([128, N], bf, tag="xp", name="xp", bufs=2) for _ in range(2)]
        for k in range(2):
            r0 = 128 * k
            nc.sync.dma_start(xw_t[k][:, 0:HLOC], xw_d[r0:r0 + 128, 0:HLOC])
        for k in range(2):
            r0 = 128 * k
            nc.sync.dma_start(xw_t[k][:, HLOC:XWC], xw_d[r0:r0 + 128, HLOC:XWC])
        for k in range(2):
            r0 = 128 * k
            nc.scalar.dma_start(xp_t[k][:, :], xp_d[r0:r0 + 128, :])
        # wgapT/wT: [1024, 256] -> [128, (g=2*si+k) * 256]
        wgap_t = sb_w.tile([128, 8 * C], bf, tag="wgap", name="wgap")
        wt_t = sb_w.tile([128, 8 * C], bf, tag="wt", name="wt")
        nc.scalar.dma_start(wgap_t[:, :].rearrange("p (g c) -> p g c", g=8),
                            wgapT_d.rearrange("(g p) c -> p g c", p=128))
        nc.scalar.dma_start(wt_t[:, :].rearrange("p (g c) -> p g c", g=8),
                            wT_d.rearrange("(g p) c -> p g c", p=128))
        wsum_t = [sb_w.tile([128, C], bf, tag="wsum", name="wsum", bufs=2) for _ in range(2)]
        for k in range(2):
            nc.scalar.dma_start(wsum_t[k][:, :], wsumT_d[128 * k:128 * (k + 1), :])

        # ---- small SBUF tiles ----
        negones = sb_s.tile([128, 1], bf, tag="ones", name="ones")
        nc.vector.memset(negones[:, :], -1.0)
        # dummy activation: pulls ACT_TABLE_LOAD (1.3us) off the critical path
        scratch = sb_s.tile([128, 1], dt.float32, tag="scr", name="scr")
        nc.scalar.activation(scratch[:, :], negones[:, :], AF.Exp)
        sq_t = [sb_s.tile([128, HLOC], bf, tag="sq", name="sq", bufs=2) for _ in range(2)]
        for k in range(2):
            nc.vector.tensor_mul(sq_t[k][:, :], xw_t[k][:, 0:HLOC], xw_t[k][:, 0:HLOC])
        ndiag = sb_s.tile([128, NBL], dt.float32, tag="ndiag", name="ndiag")
        pool_f = [sb_s.tile([128, S2TOT], dt.float32, tag="poolf", name="poolf", bufs=2) for _ in range(2)]
        pool_b = [sb_s.tile([128, S2TOT], bf, tag="poolb", name="poolb", bufs=2) for _ in range(2)]
        xcT = sb_s.tile([128, NBL * S2TOT], bf, tag="xcT", name="xcT")
        xcs = sb_s.tile([128, NBL * S2TOT], bf, tag="xcs", name="xcs")
        rs = sb_s.tile([128, NBL], dt.float32, tag="rs", name="rs")
        rs5 = sb_s.tile([128, 5 * NBL], dt.float32, tag="rs5", name="rs5")
        recip = sb_s.tile([128, NBL], dt.float32, tag="recip", name="recip")
        g_all = [sb_s.tile([128, S2TOT], bf, tag="gall", name="gall", bufs=2) for _ in range(2)]
        h_allT = sb_s.tile([S2TOT, C], bf, tag="hallT", name="hallT")
        mm2_s = sb_s.tile([S2TOT, N], bf, tag="mm2s", name="mm2s")
        e_t = [sb_e.tile([128, N], bf, tag="e", name="e", bufs=NBL) for _ in range(NBL)]
        out_sb = [sb_o.tile([128, N], bf, tag="outsb", name="outsb", bufs=2) for _ in range(2)]

        with tc.tile_pool(name="psS", bufs=4, space="PSUM") as psS, \
             tc.tile_pool(name="psM", bufs=1, space="PSUM") as psM:

            # ---- -diag[n] = -||x_n||^2 via sq @ (-1), straight to [128, 9]
            dps = psS.tile([128, NBL], dt.float32, tag="psS", name="dps")
            for b in range(NBL):
                for k in range(2):
                    nc.tensor.matmul(dps[:, b:b + 1],
                                     sq_t[k][:, 128 * b:128 * (b + 1)],
                                     negones[:, :],
                                     start=(k == 0), stop=(k == 1))
            # copy on the Act engine: it gates the first EXP, and the DVE
            # queue may schedule pools first
            nc.scalar.copy(ndiag[:, :], dps[:, :])

            # ---- pools (canonical layout from xp): raw window sums
            for k in range(2):
                v = xp_t[k][:, :].rearrange("c (i hp j wp) -> c i j hp wp",
                                            i=6, hp=8, j=6, wp=8)
                nc.vector.reduce_sum(
                    pool_f[k][:, 14:50].rearrange("c (i j) -> c i j", i=6),
                    v, axis=AX.XY)
                p6 = pool_f[k][:, 14:50]
                nc.vector.reduce_sum(pool_f[k][:, 0:1], p6, axis=AX.X)
                nc.vector.reduce_sum(
                    pool_f[k][:, 1:5].rearrange("c (p q) -> c p q", p=2),
                    p6.rearrange("c (p a q b) -> c p q a b", p=2, a=3, q=2, b=3),
                    axis=AX.XY)
                nc.vector.reduce_sum(
                    pool_f[k][:, 5:14].rearrange("c (p q) -> c p q", p=3),
                    p6.rearrange("c (p a q b) -> c p q a b", p=3, a=2, q=3, b=2),
                    axis=AX.XY)
                nc.vector.tensor_copy(pool_b[k][:, :], pool_f[k][:, :])

            # [50, 2048] = 4 banks; the tail region [2048:2304] accumulates
            # after the block loop in a psS-rotation tile (frees a bank for a
            # deeper scores pipeline)
            mm2ps = psM.tile([S2TOT, 2048], dt.float32, tag="psM", name="mm2ps")

            def scores_block(b):
                for ri, (c0, mw, ew) in enumerate(REGS):
                    sps = psS.tile([128, 512], dt.float32, tag="psS", name="sps")
                    for k in range(2):
                        nc.tensor.matmul(sps[:, 0:mw],
                                         xw_t[k][:, 128 * b:128 * (b + 1)],
                                         xw_t[k][:, c0:c0 + mw],
                                         start=(k == 0), stop=(k == 1))
                    # rowsum via the Act accumulator: avoids DVE re-reading
                    # e_t (SBUF port contention with the exp writes)
                    nc.scalar.activation(e_t[b][:, c0:c0 + ew], sps[:, 0:ew],
                                         AF.Exp, bias=ndiag[:, b:b + 1],
                                         accum_out=rs5[:, 5 * b + ri:5 * b + ri + 1])
                    if ew != mw:  # tail region carries folded xcT columns
                        nc.vector.tensor_copy(
                            xcT[:, S2TOT * b:S2TOT * (b + 1)], sps[:, ew:mw])
                nc.vector.reduce_sum(rs[:, b:b + 1], rs5[:, 5 * b:5 * b + 5],
                                     axis=AX.X)
                nc.vector.reciprocal(recip[:, b:b + 1], rs[:, b:b + 1])
                nc.vector.tensor_scalar_mul(
                    xcs[:, S2TOT * b:S2TOT * (b + 1)],
                    xcT[:, S2TOT * b:S2TOT * (b + 1)], recip[:, b:b + 1])

            def mm2_block(b):
                for (c0, mw) in MMREGS[:4]:
                    nc.tensor.matmul(mm2ps[:, c0:c0 + mw],
                                     xcs[:, S2TOT * b:S2TOT * (b + 1)],
                                     e_t[b][:, c0:c0 + mw],
                                     start=(b == 0), stop=(b == NBL - 1))

            def emit_g():
                gps = psS.tile([128, 2 * S2TOT], dt.float32, tag="psS", name="gps")
                for si, (S, S2, off) in enumerate(SCALES):
                    for po in range(2):
                        for k in range(2):
                            g = 2 * si + k
                            nc.tensor.matmul(
                                gps[:, S2TOT * po + off:S2TOT * po + off + S2],
                                wgap_t[:, C * g + 128 * po:C * g + 128 * (po + 1)],
                                pool_b[k][:, off:off + S2],
                                start=(k == 0), stop=(k == 1))
                for po in range(2):
                    nc.vector.tensor_scalar_max(
                        g_all[po][:, :], gps[:, S2TOT * po:S2TOT * (po + 1)], 0.0)

            def emit_h():
                # h_allT[s,:] = sum_po g[po][:, s-slice]^T @ W_S^T[po chunk]
                # two psum tiles, two scales per tile (separate column halves)
                for pair in ((3, 2), (1, 0)):
                    hps = psS.tile([36, 512], dt.float32, tag="psS", name="hps")
                    hsb = sb_s.tile([36, 512], bf, tag="hsb", name="hsb", bufs=2)
                    for idx, si in enumerate(pair):
                        S, S2, off = SCALES[si]
                        for po in range(2):
                            g = 2 * si + po
                            nc.tensor.matmul(
                                hps[0:S2, 256 * idx:256 * idx + C],
                                g_all[po][:, off:off + S2],
                                wt_t[:, C * g:C * (g + 1)],
                                start=(po == 0), stop=(po == 1))
                    for idx, si in enumerate(pair):
                        S, S2, off = SCALES[si]
                        nc.vector.tensor_copy(hsb[0:S2, 256 * idx:256 * idx + C],
                                              hps[0:S2, 256 * idx:256 * idx + C])
                        # partition-offset write: DMA (engines need 32-aligned
                        # partition bases, DMA descriptors do not)
                        nc.sync.dma_start(h_allT[off:off + S2, :],
                                            hsb[0:S2, 256 * idx:256 * idx + C])

            scores_block(0)
            scores_block(1)
            for b in range(2, NBL):
                scores_block(b)
                if b == 4:
                    emit_g()
                elif b == 5:
                    emit_h()
                mm2_block(b - 2)
            mm2_block(NBL - 2)
            mm2_block(NBL - 1)
            mm2t = psS.tile([S2TOT, 256], dt.float32, tag="psS", name="mm2t")
            for b in range(NBL):
                nc.tensor.matmul(mm2t[:, :],
                                 xcs[:, S2TOT * b:S2TOT * (b + 1)],
                                 e_t[b][:, 2048:N],
                                 start=(b == 0), stop=(b == NBL - 1))
            NCH = 384
            for c6 in range(3):
                c0 = NCH * c6
                nc.scalar.copy(mm2_s[:, c0:c0 + NCH], mm2ps[:, c0:c0 + NCH])
            for c0, cw in ((1152, 448), (1600, 448)):
                nc.vector.tensor_copy(mm2_s[:, c0:c0 + cw], mm2ps[:, c0:c0 + cw])
            nc.vector.tensor_copy(mm2_s[:, 2048:N], mm2t[:, :])

            # ---- output head inside the psS scope: the scheduler can
            # interleave head matmuls with the last mm2 accumulations
            for c3 in range(3):  # own half (rot cols [0:1152])
                c0 = NCH * c3
                for po in range(2):
                    ops = psS.tile([128, NCH], dt.float32, tag="psS", name="ops")
                    for k in range(2):
                        nc.tensor.matmul(ops[:, :],
                                         wsum_t[k][:, 128 * po:128 * (po + 1)],
                                         xw_t[k][:, c0:c0 + NCH],
                                         start=(k == 0), stop=False)
                    nc.tensor.matmul(ops[:, :],
                                     h_allT[:, 128 * po:128 * (po + 1)],
                                     mm2_s[:, c0:c0 + NCH],
                                     start=False, stop=True)
                    nc.scalar.copy(out_sb[po][:, c0:c0 + NCH], ops[:, :])
            for po in range(2):
                nc.sync.dma_start(out_d[128 * po:128 * (po + 1), 0:HLOC],
                                  out_sb[po][:, 0:HLOC])
            for c3 in range(3, 6):  # other half: h_all @ mm2 only
                c0 = NCH * c3
                for po in range(2):
                    ops = psS.tile([128, NCH], dt.float32, tag="psS", name="ops")
                    nc.tensor.matmul(ops[:, :],
                                     h_allT[:, 128 * po:128 * (po + 1)],
                                     mm2_s[:, c0:c0 + NCH],
                                     start=True, stop=True)
                    nc.vector.tensor_copy(out_sb[po][:, c0:c0 + NCH], ops[:, :])
            for po in range(2):
                nc.sync.dma_start(out_d[128 * po:128 * (po + 1), HLOC:N],
                                  out_sb[po][:, HLOC:N])


# ---------------------------------------------------------------------------
# host side
# ---------------------------------------------------------------------------

_CACHE = {}


def _prep_weights(inp):
    wattnT = np.ascontiguousarray(np.concatenate(
        [inp["w_attn1"], inp["w_attn2"], inp["w_attn3"], inp["w_attn6"]],
        0).T, np.float32)                                         # [256, 50]
    wins = {1: 2304.0, 2: 576.0, 3: 256.0, 6: 64.0}
    wgapT = np.concatenate(
        [np.asarray(inp[f"w_gap{S}"], np.float32).T / wins[S]
         for S in (1, 2, 3, 6)], 0)                               # [1024, 256]
    wf = np.asarray(inp["w_final"], np.float32)
    Wb = [wf[:, i * C:(i + 1) * C] for i in range(5)]
    wT = np.concatenate([Wb[1].T, Wb[2].T, Wb[3].T, Wb[4].T], 0)  # [1024, 256]
    wsumT = (Wb[0] + Wb[1] + Wb[2] + Wb[3] + Wb[4]).T             # [256, 256]
    return wattnT, wgapT, wT, wsumT


def _build_nc():
    nc = bacc.Bacc("TRN2", target_bir_lowering=False, debug=False, num_devices=8)
    bf = dt.bfloat16
    ins = {
        "xw": nc.dram_tensor("xw", [C, XWC], bf, kind="ExternalInput").ap(),
        "xp": nc.dram_tensor("xp", [C, N], bf, kind="ExternalInput").ap(),
        "wgapT": nc.dram_tensor("wgapT", [4 * C, C], bf, kind="ExternalInput").ap(),
        "wT": nc.dram_tensor("wT", [4 * C, C], bf, kind="ExternalInput").ap(),
        "wsumT": nc.dram_tensor("wsumT", [C, C], bf, kind="ExternalInput").ap(),
    }
    outs = {"out": nc.dram_tensor("out", [C, N], bf, kind="ExternalOutput").ap()}
    with tile.TileContext(nc) as tc:
        build_lspm(tc, outs, ins)
    nc.compile()
    return nc


def _in_maps(inp):
    import ml_dtypes
    bf = ml_dtypes.bfloat16
    wattnT, wgapT, wT, wsumT = _prep_weights(inp)
    wgapT_b = np.ascontiguousarray(wgapT.astype(bf))
    wT_b = np.ascontiguousarray(wT.astype(bf))
    wsumT_b = np.ascontiguousarray(wsumT.astype(bf))
    x = np.asarray(inp["x"], np.float32)
    maps = []
    for core in range(8):
        b, h = core // 2, core % 2
        xf = x[b].reshape(C, N)
        xrot = np.roll(xf, -HLOC * h, axis=1)
        xw = np.ascontiguousarray(
            np.concatenate([xrot, wattnT], 1).astype(bf))
        maps.append({"xw": xw, "xp": np.ascontiguousarray(xf.astype(bf)),
                     "wgapT": wgapT_b, "wT": wT_b, "wsumT": wsumT_b})
    return maps


def run(inputs, trace=False, **kw):
    if "nc" not in _CACHE:
        _CACHE["nc"] = _build_nc()
    nc = _CACHE["nc"]
    res = bass_utils.run_bass_kernel_spmd(
        nc, _in_maps(inputs), core_ids=list(range(8)), trace=trace, **kw)
    out = np.empty((B, C, N), np.float32)
    for b in range(B):
        pa = np.asarray(res.results[2 * b]["out"], dtype=np.float32)
        pb = np.asarray(res.results[2 * b + 1]["out"], dtype=np.float32)
        out[b] = pa + np.roll(pb, HLOC, axis=1)
    return out.reshape(B, C, H, W), res


def kernel(**inputs) -> np.ndarray:
    out, _ = run(inputs, trace=False)
    return out


# revision 37
# speedup vs baseline: 1.0144x; 1.0144x over previous
"""Trainium2 Bass kernel for LSPM (nn_LSPM_41455024341635).

Math refactor (same identities as the validated baseline):
  - scores = xf^T xf and softmax(scores) are identical for all 4 LSPM scales
    -> computed once per sample.
  - softmax shift uses diag[n] = ||x_n||^2 (shift-invariant, cancels exactly
    after row normalization).
  - row normalization folds into the mm2 lhs: xcs[n,s] = xcT[n,s]/rowsum[n].
  - all 1x1 convs + residuals fold into the output head:
      out = Wsum @ xf + h_all @ mm2,  Wsum = sum of w_final C-blocks,
      h_S = W_S @ relu(w_gap_S @ poolsum_S / win_S), h_all = concat_S h_S.
  - adaptive pools are raw window SUMS on device; the 1/win_S scale is folded
    into w_gap on the host. pool1/2/3 derive from pool6 block sums.

Sharding (collective-free): 8 cores = 4 samples x 2 attention ROW-halves.
Each core computes softmax rows n in its half (rowsum is local -> no
cross-core reduction) and produces a FULL-WIDTH partial output
  Y_h = [Wsum @ xf](own column half) + h_all @ mm2_h          [C, N] bf16
where mm2_h sums over the core's rows only. The host adds the two partials
(the head is linear in mm2). The row-half is presented to the one shared
program by pre-rolling x columns on the host (h=1 cores see x rolled by
N/2); pools read a separate canonical copy xp since a 24-row roll is not
pool3-window aligned. Host un-rolls the partial outputs before adding.

All matmuls run in bf16 (1 row/cycle on PE); exp on the Act engine per
512-col PSUM region; xcT is folded into the last scores region (free on PE).
"""

import os
import sys
import numpy as np

for _p in ("/opt/trn_rl_repo", "/root/.axon_site/_ro/trn_rl_repo"):
    if os.path.isdir(_p) and _p not in sys.path:
        sys.path.insert(0, _p)

import concourse.bass as bass
import concourse.bacc as bacc
import concourse.mybir as mybir
import concourse.tile as tile
from concourse import bass_utils

dt = mybir.dt
AF = mybir.ActivationFunctionType
AX = mybir.AxisListType

B, C, H, W = 4, 256, 48, 48
N = H * W            # 2304
HLOC = N // 2        # 1152 local attention rows per core
NBL = HLOC // 128    # 9 local row blocks
S2TOT = 50
SCALES = ((1, 1, 0), (2, 4, 1), (3, 9, 5), (6, 36, 14))  # (S, S2, col offset)
XWC = N + S2TOT      # 2354: x columns + folded wattnT columns
# scores regions per block: 4 x 512 + tail 306 (256 scores + 50 xcT)
REGS = ((0, 512, 512), (512, 512, 512), (1024, 512, 512), (1536, 512, 512),
        (2048, 306, 256))  # (col0, matmul width, exp width)
MMREGS = ((0, 512), (512, 512), (1024, 512), (1536, 512), (2048, 256))


def build_lspm(tc, outs, ins):
    nc = tc.nc
    xw_d, xp_d = ins["xw"], ins["xp"]
    wgapT_d, wT_d, wsumT_d = ins["wgapT"], ins["wT"], ins["wsumT"]
    out_d = outs["out"]
    bf = dt.bfloat16

    from contextlib import ExitStack
    with ExitStack() as ctx:
        pool = lambda name, bufs: ctx.enter_context(
            tc.tile_pool(name=name, bufs=bufs))
        sb_x = pool("x", 1)
        sb_e = pool("e", 1)
        sb_w = pool("w", 1)
        sb_s = pool("s", 1)
        sb_o = pool("o", 1)

        # ---- input DMAs ----
        # critical x loads trigger from the SP queue; the rest from the Act
        # queue (each trigger costs ~0.6-1.6us on its issuing sequencer, and
        # only SP/Act/Pool may trigger DMAs)
        xw_t = [sb_x.tile([128, XWC], bf, tag="xw", name="xw", bufs=2) for _ in range(2)]
        xp_t = [sb_x.tile([128, N], bf, tag="xp", name="xp", bufs=2) for _ in range(2)]
        PW = 384
        nc.sync.dma_start(xw_t[0][:, 0:PW], xw_d[0:128, 0:PW])
        nc.sync.dma_start(xw_t[0][:, PW:2 * PW], xw_d[0:128, PW:2 * PW])
        nc.sync.dma_start(xw_t[0][:, HLOC:XWC], xw_d[0:128, HLOC:XWC])
        nc.sync.dma_start(xw_t[0][:, 2 * PW:HLOC], xw_d[0:128, 2 * PW:HLOC])
        nc.scalar.dma_start(xw_t[1][:, 0:PW], xw_d[128:256, 0:PW])
        nc.scalar.dma_start(xw_t[1][:, PW:2 * PW], xw_d[128:256, PW:2 * PW])
        nc.scalar.dma_start(xw_t[1][:, HLOC:XWC], xw_d[128:256, HLOC:XWC])
        nc.scalar.dma_start(xw_t[1][:, 2 * PW:HLOC], xw_d[128:256, 2 * PW:HLOC])
        for k in range(2):
            r0 = 128 * k
            nc.sync.dma_start(xp_t[k][:, :], xp_d[r0:r0 + 128, :])
        # wgapT/wT: [1024, 256] -> [128, (g=2*si+k) * 256]
        wgap_t = sb_w.tile([128, 8 * C], bf, tag="wgap", name="wgap")
        wt_t = sb_w.tile([128, 8 * C], bf, tag="wt", name="wt")
        nc.sync.dma_start(wgap_t[:, :].rearrange("p (g c) -> p g c", g=8),
                            wgapT_d.rearrange("(g p) c -> p g c", p=128))
        nc.sync.dma_start(wt_t[:, :].rearrange("p (g c) -> p g c", g=8),
                            wT_d.rearrange("(g p) c -> p g c", p=128))
        wsum_t = [sb_w.tile([128, C], bf, tag="wsum", name="wsum", bufs=2) for _ in range(2)]
        for k in range(2):
            nc.sync.dma_start(wsum_t[k][:, :], wsumT_d[128 * k:128 * (k + 1), :])

        # ---- small SBUF tiles ----
        negones = sb_s.tile([128, 1], bf, tag="ones", name="ones")
        nc.vector.memset(negones[:, :], -1.0)
        # dummy activation: pulls ACT_TABLE_LOAD (1.3us) off the critical path
        scratch = sb_s.tile([128, 1], dt.float32, tag="scr", name="scr")
        nc.scalar.activation(scratch[:, :], negones[:, :], AF.Exp)
        sq_t = [sb_s.tile([128, HLOC], bf, tag="sq", name="sq", bufs=2) for _ in range(2)]
        for p in range(3):
            for k in range(2):
                nc.vector.tensor_mul(sq_t[k][:, PW * p:PW * (p + 1)],
                                     xw_t[k][:, PW * p:PW * (p + 1)],
                                     xw_t[k][:, PW * p:PW * (p + 1)])
        ndiag = sb_s.tile([128, NBL], dt.float32, tag="ndiag", name="ndiag")
        pool_f = [sb_s.tile([128, S2TOT], dt.float32, tag="poolf", name="poolf", bufs=2) for _ in range(2)]
        pool_b = [sb_s.tile([128, S2TOT], bf, tag="poolb", name="poolb", bufs=2) for _ in range(2)]
        xcT = sb_s.tile([128, NBL * S2TOT], bf, tag="xcT", name="xcT")
        xcs = sb_s.tile([128, NBL * S2TOT], bf, tag="xcs", name="xcs")
        rs = sb_s.tile([128, NBL], dt.float32, tag="rs", name="rs")
        rs5 = sb_s.tile([128, 5 * NBL], dt.float32, tag="rs5", name="rs5")
        recip = sb_s.tile([128, NBL], dt.float32, tag="recip", name="recip")
        g_all = [sb_s.tile([128, S2TOT], bf, tag="gall", name="gall", bufs=2) for _ in range(2)]
        h_allT = sb_s.tile([S2TOT, C], bf, tag="hallT", name="hallT")
        mm2_s = sb_s.tile([S2TOT, N], bf, tag="mm2s", name="mm2s")
        e_t = [sb_e.tile([128, N], bf, tag="e", name="e", bufs=NBL) for _ in range(NBL)]
        out_sb = [sb_o.tile([128, N], bf, tag="outsb", name="outsb", bufs=2) for _ in range(2)]

        with tc.tile_pool(name="psS", bufs=4, space="PSUM") as psS, \
             tc.tile_pool(name="psM", bufs=1, space="PSUM") as psM:

            # ---- -diag[n] = -||x_n||^2 via sq @ (-1), straight to [128, 9]
            dps = psS.tile([128, NBL], dt.float32, tag="psS", name="dps")
            for p in range(3):  # grouped by sq piece to pipeline the startup
                for b in range(3 * p, 3 * p + 3):
                    for k in range(2):
                        nc.tensor.matmul(dps[:, b:b + 1],
                                         sq_t[k][:, 128 * b:128 * (b + 1)],
                                         negones[:, :],
                                         start=(k == 0), stop=(k == 1))
            # copies on the Act engine (gates the first EXP); split so early
            # blocks' exps don't wait for the last diag group
            nc.scalar.copy(ndiag[:, 0:6], dps[:, 0:6])
            nc.scalar.copy(ndiag[:, 6:NBL], dps[:, 6:NBL])

            # ---- pools (canonical layout from xp): raw window sums
            for k in range(2):
                v = xp_t[k][:, :].rearrange("c (i hp j wp) -> c i j hp wp",
                                            i=6, hp=8, j=6, wp=8)
                nc.vector.reduce_sum(
                    pool_f[k][:, 14:50].rearrange("c (i j) -> c i j", i=6),
                    v, axis=AX.XY)
                p6 = pool_f[k][:, 14:50]
                nc.vector.reduce_sum(pool_f[k][:, 0:1], p6, axis=AX.X)
                nc.vector.reduce_sum(
                    pool_f[k][:, 1:5].rearrange("c (p q) -> c p q", p=2),
                    p6.rearrange("c (p a q b) -> c p q a b", p=2, a=3, q=2, b=3),
                    axis=AX.XY)
                nc.vector.reduce_sum(
                    pool_f[k][:, 5:14].rearrange("c (p q) -> c p q", p=3),
                    p6.rearrange("c (p a q b) -> c p q a b", p=3, a=2, q=3, b=2),
                    axis=AX.XY)
                nc.vector.tensor_copy(pool_b[k][:, :], pool_f[k][:, :])

            # [50, 2048] = 4 banks; the tail region [2048:2304] accumulates
            # after the block loop in a psS-rotation tile (frees a bank for a
            # deeper scores pipeline)
            mm2ps = psM.tile([S2TOT, 2048], dt.float32, tag="psM", name="mm2ps")

            def scores_block(b):
                for ri, (c0, mw, ew) in enumerate(REGS):
                    sps = psS.tile([128, 512], dt.float32, tag="psS", name="sps")
                    for k in range(2):
                        nc.tensor.matmul(sps[:, 0:mw],
                                         xw_t[k][:, 128 * b:128 * (b + 1)],
                                         xw_t[k][:, c0:c0 + mw],
                                         start=(k == 0), stop=(k == 1))
                    # rowsum via the Act accumulator: avoids DVE re-reading
                    # e_t (SBUF port contention with the exp writes)
                    nc.scalar.activation(e_t[b][:, c0:c0 + ew], sps[:, 0:ew],
                                         AF.Exp, bias=ndiag[:, b:b + 1],
                                         accum_out=rs5[:, 5 * b + ri:5 * b + ri + 1])
                    if ew != mw:  # tail region carries folded xcT columns
                        nc.vector.tensor_copy(
                            xcT[:, S2TOT * b:S2TOT * (b + 1)], sps[:, ew:mw])
                nc.vector.reduce_sum(rs[:, b:b + 1], rs5[:, 5 * b:5 * b + 5],
                                     axis=AX.X)
                nc.vector.reciprocal(recip[:, b:b + 1], rs[:, b:b + 1])
                nc.vector.tensor_scalar_mul(
                    xcs[:, S2TOT * b:S2TOT * (b + 1)],
                    xcT[:, S2TOT * b:S2TOT * (b + 1)], recip[:, b:b + 1])

            def mm2_block(b):
                for (c0, mw) in MMREGS[:4]:
                    nc.tensor.matmul(mm2ps[:, c0:c0 + mw],
                                     xcs[:, S2TOT * b:S2TOT * (b + 1)],
                                     e_t[b][:, c0:c0 + mw],
                                     start=(b == 0), stop=(b == NBL - 1))

            def emit_g():
                gps = psS.tile([128, 2 * S2TOT], dt.float32, tag="psS", name="gps")
                for si, (S, S2, off) in enumerate(SCALES):
                    for po in range(2):
                        for k in range(2):
                            g = 2 * si + k
                            nc.tensor.matmul(
                                gps[:, S2TOT * po + off:S2TOT * po + off + S2],
                                wgap_t[:, C * g + 128 * po:C * g + 128 * (po + 1)],
                                pool_b[k][:, off:off + S2],
                                start=(k == 0), stop=(k == 1))
                for po in range(2):
                    nc.vector.tensor_scalar_max(
                        g_all[po][:, :], gps[:, S2TOT * po:S2TOT * (po + 1)], 0.0)

            def emit_h():
                # h_allT[s,:] = sum_po g[po][:, s-slice]^T @ W_S^T[po chunk]
                # two psum tiles, two scales per tile (separate column halves)
                for pair in ((3, 2), (1, 0)):
                    hps = psS.tile([36, 512], dt.float32, tag="psS", name="hps")
                    hsb = sb_s.tile([36, 512], bf, tag="hsb", name="hsb", bufs=2)
                    for idx, si in enumerate(pair):
                        S, S2, off = SCALES[si]
                        for po in range(2):
                            g = 2 * si + po
                            nc.tensor.matmul(
                                hps[0:S2, 256 * idx:256 * idx + C],
                                g_all[po][:, off:off + S2],
                                wt_t[:, C * g:C * (g + 1)],
                                start=(po == 0), stop=(po == 1))
                    for idx, si in enumerate(pair):
                        S, S2, off = SCALES[si]
                        nc.vector.tensor_copy(hsb[0:S2, 256 * idx:256 * idx + C],
                                              hps[0:S2, 256 * idx:256 * idx + C])
                        # partition-offset write: DMA (engines need 32-aligned
                        # partition bases, DMA descriptors do not)
                        nc.sync.dma_start(h_allT[off:off + S2, :],
                                            hsb[0:S2, 256 * idx:256 * idx + C])

            scores_block(0)
            scores_block(1)
            for b in range(2, NBL):
                scores_block(b)
                if b == 4:
                    emit_g()
                elif b == 5:
                    emit_h()
                mm2_block(b - 2)
            mm2_block(NBL - 2)
            mm2_block(NBL - 1)
            mm2t = psS.tile([S2TOT, 256], dt.float32, tag="psS", name="mm2t")
            for b in range(NBL):
                nc.tensor.matmul(mm2t[:, :],
                                 xcs[:, S2TOT * b:S2TOT * (b + 1)],
                                 e_t[b][:, 2048:N],
                                 start=(b == 0), stop=(b == NBL - 1))
            NCH = 384
            for c6 in range(3):
                c0 = NCH * c6
                nc.scalar.copy(mm2_s[:, c0:c0 + NCH], mm2ps[:, c0:c0 + NCH])
            for c0, cw in ((1152, 448), (1600, 448)):
                nc.vector.tensor_copy(mm2_s[:, c0:c0 + cw], mm2ps[:, c0:c0 + cw])
            nc.vector.tensor_copy(mm2_s[:, 2048:N], mm2t[:, :])

            # ---- output head inside the psS scope: the scheduler can
            # interleave head matmuls with the last mm2 accumulations
            for c3 in range(3):  # own half (rot cols [0:1152])
                c0 = NCH * c3
                for po in range(2):
                    ops = psS.tile([128, NCH], dt.float32, tag="psS", name="ops")
                    for k in range(2):
                        nc.tensor.matmul(ops[:, :],
                                         wsum_t[k][:, 128 * po:128 * (po + 1)],
                                         xw_t[k][:, c0:c0 + NCH],
                                         start=(k == 0), stop=False)
                    nc.tensor.matmul(ops[:, :],
                                     h_allT[:, 128 * po:128 * (po + 1)],
                                     mm2_s[:, c0:c0 + NCH],
                                     start=False, stop=True)
                    nc.scalar.copy(out_sb[po][:, c0:c0 + NCH], ops[:, :])
            for po in range(2):
                nc.sync.dma_start(out_d[128 * po:128 * (po + 1), 0:HLOC],
                                  out_sb[po][:, 0:HLOC])
            for c3 in range(3, 6):  # other half: h_all @ mm2 only
                c0 = NCH * c3
                for po in range(2):
                    ops = psS.tile([128, NCH], dt.float32, tag="psS", name="ops")
                    nc.tensor.matmul(ops[:, :],
                                     h_allT[:, 128 * po:128 * (po + 1)],
                                     mm2_s[:, c0:c0 + NCH],
                                     start=True, stop=True)
                    nc.vector.tensor_copy(out_sb[po][:, c0:c0 + NCH], ops[:, :])
            for po in range(2):
                nc.sync.dma_start(out_d[128 * po:128 * (po + 1), HLOC:N],
                                  out_sb[po][:, HLOC:N])


# ---------------------------------------------------------------------------
# host side
# ---------------------------------------------------------------------------

_CACHE = {}


def _prep_weights(inp):
    wattnT = np.ascontiguousarray(np.concatenate(
        [inp["w_attn1"], inp["w_attn2"], inp["w_attn3"], inp["w_attn6"]],
        0).T, np.float32)                                         # [256, 50]
    wins = {1: 2304.0, 2: 576.0, 3: 256.0, 6: 64.0}
    wgapT = np.concatenate(
        [np.asarray(inp[f"w_gap{S}"], np.float32).T / wins[S]
         for S in (1, 2, 3, 6)], 0)                               # [1024, 256]
    wf = np.asarray(inp["w_final"], np.float32)
    Wb = [wf[:, i * C:(i + 1) * C] for i in range(5)]
    wT = np.concatenate([Wb[1].T, Wb[2].T, Wb[3].T, Wb[4].T], 0)  # [1024, 256]
    wsumT = (Wb[0] + Wb[1] + Wb[2] + Wb[3] + Wb[4]).T             # [256, 256]
    return wattnT, wgapT, wT, wsumT


def _build_nc():
    nc = bacc.Bacc("TRN2", target_bir_lowering=False, debug=False, num_devices=8)
    bf = dt.bfloat16
    ins = {
        "xw": nc.dram_tensor("xw", [C, XWC], bf, kind="ExternalInput").ap(),
        "xp": nc.dram_tensor("xp", [C, N], bf, kind="ExternalInput").ap(),
        "wgapT": nc.dram_tensor("wgapT", [4 * C, C], bf, kind="ExternalInput").ap(),
        "wT": nc.dram_tensor("wT", [4 * C, C], bf, kind="ExternalInput").ap(),
        "wsumT": nc.dram_tensor("wsumT", [C, C], bf, kind="ExternalInput").ap(),
    }
    outs = {"out": nc.dram_tensor("out", [C, N], bf, kind="ExternalOutput").ap()}
    with tile.TileContext(nc) as tc:
        build_lspm(tc, outs, ins)
    nc.compile()
    return nc


def _in_maps(inp):
    import ml_dtypes
    bf = ml_dtypes.bfloat16
    wattnT, wgapT, wT, wsumT = _prep_weights(inp)
    wgapT_b = np.ascontiguousarray(wgapT.astype(bf))
    wT_b = np.ascontiguousarray(wT.astype(bf))
    wsumT_b = np.ascontiguousarray(wsumT.astype(bf))
    x = np.asarray(inp["x"], np.float32)
    maps = []
    for core in range(8):
        b, h = core // 2, core % 2
        xf = x[b].reshape(C, N)
        xrot = np.roll(xf, -HLOC * h, axis=1)
        xw = np.ascontiguousarray(
            np.concatenate([xrot, wattnT], 1).astype(bf))
        maps.append({"xw": xw, "xp": np.ascontiguousarray(xf.astype(bf)),
                     "wgapT": wgapT_b, "wT": wT_b, "wsumT": wsumT_b})
    return maps


def run(inputs, trace=False, **kw):
    if "nc" not in _CACHE:
        _CACHE["nc"] = _build_nc()
    nc = _CACHE["nc"]
    res = bass_utils.run_bass_kernel_spmd(
        nc, _in_maps(inputs), core_ids=list(range(8)), trace=trace, **kw)
    out = np.empty((B, C, N), np.float32)
    for b in range(B):
        pa = np.asarray(res.results[2 * b]["out"], dtype=np.float32)
        pb = np.asarray(res.results[2 * b + 1]["out"], dtype=np.float32)
        out[b] = pa + np.roll(pb, HLOC, axis=1)
    return out.reshape(B, C, H, W), res


def kernel(**inputs) -> np.ndarray:
    out, _ = run(inputs, trace=False)
    return out
